# revision 1
# baseline (speedup 1.0000x reference)
"""AtomAttentionEncoder distributed kernel for 8 trn2 NeuronCores.

Strategy (per sharding hint): shard atoms (query dim N=512 -> 64 rows/core)
for the pair-bias MLP, the token-pair MLP, attention queries and
transitions; replicate params. Per layer: transitions + attention output
are computed on each core's own 64 rows, and the full [512,768] activation
is re-assembled with an all_gather so every core has the K/V context.
The [N,N] pair MLPs (the dominant FLOPs) shard perfectly on the first N
axis with zero communication; the attention bias rows stay core-local
because queries are sharded the same way.
"""

import numpy as np

D = 768
FF = 3072
H = 12
HD = D // H
P = 16
TS = 384
NL = 3
N = 512
NCORES = 8
ROWS = N // NCORES  # 64


def _forward_rows(rows_slice, feats, pair_rows, mask1, maskp, p, jnp, jax, axis_name):
    """Row-sharded forward. rows_slice: [ROWS] global row indices."""
    # atom conditioning, full 512 rows (cheap, replicated)
    atom_cond = feats @ p['w_atom_cond'] + p['b_atom_cond'] + p['pos_embed'][:N]

    ph = atom_cond @ p['pair_wh'] + p['pair_bh']          # [N,P]
    pw = atom_cond @ p['pair_ww'] + p['pair_bw']          # [N,P]

    # pair MLP on own rows: [ROWS, N, P]
    pf = pair_rows + ph[rows_slice][:, None, :] + pw[None, :, :]
    pf = jnp.maximum(pf @ p['pair_m1'] + p['pair_mb1'], 0.0) @ p['pair_m2'] + p['pair_mb2']

    bias_rows = (jnp.maximum(pf @ p['bias_w1'] + p['bias_b1'], 0.0)
                 @ p['bias_w2'] + p['bias_b2'])[..., 0]   # [ROWS, N]

    tp_rows = jnp.maximum(pf @ p['tp_w1'] + p['tp_b1'], 0.0) @ p['tp_w2'] + p['tp_b2']

    # mask: key mask AND query (block) mask for own rows
    attn_mask = mask1[None, None, :] & maskp[rows_slice][None, :, None]  # [1,ROWS,N]
    neg = np.float32(np.finfo(np.float32).min)
    scale = np.float32(HD ** -0.5)

    x = atom_cond  # full [N, D], replicated
    for l in range(NL):
        xr = x[rows_slice]                                 # [ROWS, D]
        m = jnp.mean(xr, axis=-1, keepdims=True)
        v = jnp.var(xr, axis=-1, keepdims=True)
        h = (xr - m) * jax.lax.rsqrt(v + 1e-5) * p['ln_g'][l] + p['ln_b'][l]
        h = h @ p['t_w1'][l] + p['t_b1'][l]
        h = h * jax.nn.sigmoid(h)                          # silu
        h = h @ p['t_w2'][l] + p['t_b2'][l]
        xr = xr + h

        # gather full post-transition x for K/V
        xg = jax.lax.all_gather(xr, axis_name, axis=0, tiled=True)  # [N, D]

        q = (xr @ p['wq'][l] + p['bq'][l]).reshape(ROWS, H, HD).transpose(1, 0, 2)
        k = (xg @ p['wk'][l] + p['bk'][l]).reshape(N, H, HD).transpose(1, 0, 2)
        vv = (xg @ p['wv'][l] + p['bv'][l]).reshape(N, H, HD).transpose(1, 0, 2)

        aw = jnp.einsum('hqd,hkd->hqk', q, k) * scale + bias_rows[None, :, :]
        aw = jnp.where(attn_mask, aw, neg)
        probs = jax.nn.softmax(aw, axis=-1)
        o = jnp.einsum('hqk,hkd->hqd', probs, vv).transpose(1, 0, 2).reshape(ROWS, D)
        xr = o @ p['wo'][l] + p['bo'][l]

        x = jax.lax.all_gather(xr, axis_name, axis=0, tiled=True)   # [N, D]

    ts_rows = x[rows_slice] @ p['w_ts'] + p['b_ts']        # [ROWS, TS]
    return x, ts_rows, tp_rows, bias_rows


def _kernel_device(atom_single_input_feats, atom_block_pair_input_feats,
                   atom_single_mask, atom_block_pair_mask,
                   atom_noised_coords, params):
    import jax
    import jax.numpy as jnp

    devs = jax.devices()[:NCORES]
    assert len(devs) == NCORES

    feats = np.asarray(atom_single_input_feats)[0]         # [N, TS]
    pair = np.asarray(atom_block_pair_input_feats)[0]      # [N, N, P]
    mask1 = np.asarray(atom_single_mask)[0]                # [N]
    maskp = np.asarray(atom_block_pair_mask)[0]            # [N]
    p = {k: np.asarray(v) for k, v in params.items()}

    # per-core shards
    pair_sh = pair.reshape(NCORES, ROWS, N, P)
    rows_idx = np.arange(N, dtype=np.int32).reshape(NCORES, ROWS)

    def percore(rows_slice, pair_rows, feats_r, mask1_r, maskp_r, params_r):
        return _forward_rows(rows_slice, feats_r, pair_rows, mask1_r, maskp_r,
                             params_r, jnp, jax, 'c')

    fn = jax.pmap(percore, axis_name='c',
                  in_axes=(0, 0, None, None, None, None),
                  out_axes=(None, 0, 0, 0), devices=devs)

    x, ts_sh, tp_sh, _ = fn(rows_idx, pair_sh, feats, mask1, maskp, p)

    x = np.asarray(x)                                      # [N, D]
    ts = np.asarray(ts_sh).reshape(N, TS)
    tp = np.asarray(tp_sh).reshape(N, N, P)
    return (x[None].astype(np.float32),
            ts[None].astype(np.float32),
            tp[None].astype(np.float32))


def _kernel_host(atom_single_input_feats, atom_block_pair_input_feats,
                 atom_single_mask, atom_block_pair_mask,
                 atom_noised_coords, params):
    """Pure numpy fallback — exact port of the reference."""
    feats = np.asarray(atom_single_input_feats, dtype=np.float32)
    pair_in = np.asarray(atom_block_pair_input_feats, dtype=np.float32)
    mask1 = np.asarray(atom_single_mask)
    maskp = np.asarray(atom_block_pair_mask)
    p = {k: np.asarray(v, dtype=np.float32) for k, v in params.items()}

    B, n, _ = feats.shape
    atom_cond = feats @ p['w_atom_cond'] + p['b_atom_cond'] + p['pos_embed'][:n]
    ph = atom_cond @ p['pair_wh'] + p['pair_bh']
    pw = atom_cond @ p['pair_ww'] + p['pair_bw']
    pf = pair_in + ph[:, :, None, :] + pw[:, None, :, :]
    pf = np.maximum(pf @ p['pair_m1'] + p['pair_mb1'], 0) @ p['pair_m2'] + p['pair_mb2']
    bias = (np.maximum(pf @ p['bias_w1'] + p['bias_b1'], 0) @ p['bias_w2'] + p['bias_b2'])[..., 0]
    attn_mask = (mask1[:, None, None, :] & maskp[:, None, :, None])
    neg = np.finfo(np.float32).min
    scale = HD ** -0.5
    x = atom_cond
    for l in range(NL):
        mu = x.mean(-1, keepdims=True)
        va = x.var(-1, keepdims=True)
        h = (x - mu) / np.sqrt(va + 1e-5) * p['ln_g'][l] + p['ln_b'][l]
        h1 = h @ p['t_w1'][l] + p['t_b1'][l]
        h1 = h1 / (1 + np.exp(-h1))
        x = x + h1 @ p['t_w2'][l] + p['t_b2'][l]
        q = (x @ p['wq'][l] + p['bq'][l]).reshape(B, n, H, HD).transpose(0, 2, 1, 3)
        k = (x @ p['wk'][l] + p['bk'][l]).reshape(B, n, H, HD).transpose(0, 2, 1, 3)
        v = (x @ p['wv'][l] + p['bv'][l]).reshape(B, n, H, HD).transpose(0, 2, 1, 3)
        aw = np.einsum('bhqd,bhkd->bhqk', q, k) * scale + bias[:, None, :, :]
        aw = np.where(attn_mask, aw, neg).astype(np.float32)
        aw = aw - aw.max(-1, keepdims=True)
        e = np.exp(aw)
        probs = e / e.sum(-1, keepdims=True)
        o = np.einsum('bhqk,bhkd->bhqd', probs, v).transpose(0, 2, 1, 3).reshape(B, n, D)
        x = o @ p['wo'][l] + p['bo'][l]
    ts_out = x @ p['w_ts'] + p['b_ts']
    tp_out = np.maximum(pf @ p['tp_w1'] + p['tp_b1'], 0) @ p['tp_w2'] + p['tp_b2']
    return (x.astype(np.float32), ts_out.astype(np.float32), tp_out.astype(np.float32))


def kernel(**inputs):
    try:
        return _kernel_device(**inputs)
    except Exception:
        import traceback
        traceback.print_exc()
        return _kernel_host(**inputs)


if __name__ == '__main__':
    pass


# revision 3
# speedup vs baseline: 1.7276x; 1.7276x over previous
"""AtomAttentionEncoder distributed kernel for 8 trn2 NeuronCores.

Strategy (per sharding hint): shard atoms (query dim N=512 -> 64 rows/core)
for the pair-bias MLP, the token-pair MLP, attention queries and
transitions; replicate params. Per layer: transitions + attention output
are computed on each core's own 64 rows, and the full [512,768] activation
is re-assembled with an all_gather so every core has the K/V context.
The [N,N] pair MLPs (the dominant FLOPs) shard perfectly on the first N
axis with zero communication; the attention bias rows stay core-local
because queries are sharded the same way.
"""

import numpy as np

D = 768
FF = 3072
H = 12
HD = D // H
P = 16
TS = 384
NL = 3
N = 512
NCORES = 8
ROWS = N // NCORES  # 64


def _forward_rows(rows_slice, feats, pair_rows, mask1, maskp, p, jnp, jax, axis_name):
    """Row-sharded forward. rows_slice: [ROWS] global row indices."""
    # atom conditioning, full 512 rows (cheap, replicated)
    atom_cond = feats @ p['w_atom_cond'] + p['b_atom_cond'] + p['pos_embed'][:N]

    ph = atom_cond @ p['pair_wh'] + p['pair_bh']          # [N,P]
    pw = atom_cond @ p['pair_ww'] + p['pair_bw']          # [N,P]

    # pair MLP on own rows: [ROWS, N, P]
    pf = pair_rows + ph[rows_slice][:, None, :] + pw[None, :, :]
    pf = jnp.maximum(pf @ p['pair_m1'] + p['pair_mb1'], 0.0) @ p['pair_m2'] + p['pair_mb2']

    bias_rows = (jnp.maximum(pf @ p['bias_w1'] + p['bias_b1'], 0.0)
                 @ p['bias_w2'] + p['bias_b2'])[..., 0]   # [ROWS, N]

    tp_rows = jnp.maximum(pf @ p['tp_w1'] + p['tp_b1'], 0.0) @ p['tp_w2'] + p['tp_b2']

    # mask: key mask AND query (block) mask for own rows
    attn_mask = mask1[None, None, :] & maskp[rows_slice][None, :, None]  # [1,ROWS,N]
    neg = np.float32(np.finfo(np.float32).min)
    scale = np.float32(HD ** -0.5)

    x = atom_cond  # full [N, D], replicated
    for l in range(NL):
        xr = x[rows_slice]                                 # [ROWS, D]
        m = jnp.mean(xr, axis=-1, keepdims=True)
        v = jnp.var(xr, axis=-1, keepdims=True)
        h = (xr - m) * jax.lax.rsqrt(v + 1e-5) * p['ln_g'][l] + p['ln_b'][l]
        h = h @ p['t_w1'][l] + p['t_b1'][l]
        h = h * jax.nn.sigmoid(h)                          # silu
        h = h @ p['t_w2'][l] + p['t_b2'][l]
        xr = xr + h

        # gather full post-transition x for K/V
        xg = jax.lax.all_gather(xr, axis_name, axis=0, tiled=True)  # [N, D]

        q = (xr @ p['wq'][l] + p['bq'][l]).reshape(ROWS, H, HD).transpose(1, 0, 2)
        k = (xg @ p['wk'][l] + p['bk'][l]).reshape(N, H, HD).transpose(1, 0, 2)
        vv = (xg @ p['wv'][l] + p['bv'][l]).reshape(N, H, HD).transpose(1, 0, 2)

        aw = jnp.einsum('hqd,hkd->hqk', q, k) * scale + bias_rows[None, :, :]
        aw = jnp.where(attn_mask, aw, neg)
        probs = jax.nn.softmax(aw, axis=-1)
        o = jnp.einsum('hqk,hkd->hqd', probs, vv).transpose(1, 0, 2).reshape(ROWS, D)
        xr = o @ p['wo'][l] + p['bo'][l]

        x = jax.lax.all_gather(xr, axis_name, axis=0, tiled=True)   # [N, D]

    ts_rows = x[rows_slice] @ p['w_ts'] + p['b_ts']        # [ROWS, TS]
    return x, ts_rows, tp_rows, bias_rows


_FN_CACHE = {}


def _get_fn():
    import jax
    import jax.numpy as jnp

    if 'fn' in _FN_CACHE:
        return _FN_CACHE['fn']

    devs = jax.devices()[:NCORES]
    assert len(devs) == NCORES

    def percore(rows_slice, pair_rows, feats_r, mask1_r, maskp_r, params_r):
        return _forward_rows(rows_slice, feats_r, pair_rows, mask1_r, maskp_r,
                             params_r, jnp, jax, 'c')

    fn = jax.pmap(percore, axis_name='c',
                  in_axes=(0, 0, None, None, None, None),
                  out_axes=(None, 0, 0, 0), devices=devs)
    _FN_CACHE['fn'] = fn
    return fn


def _kernel_device(atom_single_input_feats, atom_block_pair_input_feats,
                   atom_single_mask, atom_block_pair_mask,
                   atom_noised_coords, params):
    import jax
    import jax.numpy as jnp

    feats = np.asarray(atom_single_input_feats)[0]         # [N, TS]
    pair = np.asarray(atom_block_pair_input_feats)[0]      # [N, N, P]
    mask1 = np.asarray(atom_single_mask)[0]                # [N]
    maskp = np.asarray(atom_block_pair_mask)[0]            # [N]
    p = {k: np.asarray(v) for k, v in params.items()}

    # per-core shards
    pair_sh = pair.reshape(NCORES, ROWS, N, P)
    rows_idx = np.arange(N, dtype=np.int32).reshape(NCORES, ROWS)

    fn = _get_fn()
    x, ts_sh, tp_sh, _ = fn(rows_idx, pair_sh, feats, mask1, maskp, p)

    x = np.asarray(x)                                      # [N, D]
    ts = np.asarray(ts_sh).reshape(N, TS)
    tp = np.asarray(tp_sh).reshape(N, N, P)
    return (x[None].astype(np.float32),
            ts[None].astype(np.float32),
            tp[None].astype(np.float32))


def _kernel_host(atom_single_input_feats, atom_block_pair_input_feats,
                 atom_single_mask, atom_block_pair_mask,
                 atom_noised_coords, params):
    """Pure numpy fallback — exact port of the reference."""
    feats = np.asarray(atom_single_input_feats, dtype=np.float32)
    pair_in = np.asarray(atom_block_pair_input_feats, dtype=np.float32)
    mask1 = np.asarray(atom_single_mask)
    maskp = np.asarray(atom_block_pair_mask)
    p = {k: np.asarray(v, dtype=np.float32) for k, v in params.items()}

    B, n, _ = feats.shape
    atom_cond = feats @ p['w_atom_cond'] + p['b_atom_cond'] + p['pos_embed'][:n]
    ph = atom_cond @ p['pair_wh'] + p['pair_bh']
    pw = atom_cond @ p['pair_ww'] + p['pair_bw']
    pf = pair_in + ph[:, :, None, :] + pw[:, None, :, :]
    pf = np.maximum(pf @ p['pair_m1'] + p['pair_mb1'], 0) @ p['pair_m2'] + p['pair_mb2']
    bias = (np.maximum(pf @ p['bias_w1'] + p['bias_b1'], 0) @ p['bias_w2'] + p['bias_b2'])[..., 0]
    attn_mask = (mask1[:, None, None, :] & maskp[:, None, :, None])
    neg = np.finfo(np.float32).min
    scale = HD ** -0.5
    x = atom_cond
    for l in range(NL):
        mu = x.mean(-1, keepdims=True)
        va = x.var(-1, keepdims=True)
        h = (x - mu) / np.sqrt(va + 1e-5) * p['ln_g'][l] + p['ln_b'][l]
        h1 = h @ p['t_w1'][l] + p['t_b1'][l]
        h1 = h1 / (1 + np.exp(-h1))
        x = x + h1 @ p['t_w2'][l] + p['t_b2'][l]
        q = (x @ p['wq'][l] + p['bq'][l]).reshape(B, n, H, HD).transpose(0, 2, 1, 3)
        k = (x @ p['wk'][l] + p['bk'][l]).reshape(B, n, H, HD).transpose(0, 2, 1, 3)
        v = (x @ p['wv'][l] + p['bv'][l]).reshape(B, n, H, HD).transpose(0, 2, 1, 3)
        aw = np.einsum('bhqd,bhkd->bhqk', q, k) * scale + bias[:, None, :, :]
        aw = np.where(attn_mask, aw, neg).astype(np.float32)
        aw = aw - aw.max(-1, keepdims=True)
        e = np.exp(aw)
        probs = e / e.sum(-1, keepdims=True)
        o = np.einsum('bhqk,bhkd->bhqd', probs, v).transpose(0, 2, 1, 3).reshape(B, n, D)
        x = o @ p['wo'][l] + p['bo'][l]
    ts_out = x @ p['w_ts'] + p['b_ts']
    tp_out = np.maximum(pf @ p['tp_w1'] + p['tp_b1'], 0) @ p['tp_w2'] + p['tp_b2']
    return (x.astype(np.float32), ts_out.astype(np.float32), tp_out.astype(np.float32))


def kernel(**inputs):
    try:
        return _kernel_device(**inputs)
    except Exception:
        import traceback
        traceback.print_exc()
        return _kernel_host(**inputs)


if __name__ == '__main__':
    pass


# revision 8
# speedup vs baseline: 3.2589x; 1.8863x over previous
"""AtomAttentionEncoder distributed kernel for 8 trn2 NeuronCores.

Strategy (per sharding hint): shard atoms (query dim N=512 -> 64 rows/core)
for the pair-bias MLP, the token-pair MLP, attention queries and
transitions; replicate params. Per layer: transitions + attention output
are computed on each core's own 64 rows, and the full [512,768] activation
is re-assembled with an all_gather so every core has the K/V context.
The [N,N] pair MLPs (the dominant FLOPs) shard perfectly on the first N
axis with zero communication; the attention bias rows stay core-local
because queries are sharded the same way.
"""

import numpy as np

D = 768
FF = 3072
H = 12
HD = D // H
P = 16
TS = 384
NL = 3
N = 512
NCORES = 8
ROWS = N // NCORES  # 64


def _mm(x, w, jnp):
    """Matmul where w may be bf16: run the PE in bf16, accumulate fp32."""
    if w.dtype == jnp.bfloat16:
        return jnp.einsum('nd,df->nf', x.astype(jnp.bfloat16), w,
                          preferred_element_type=jnp.float32)
    return x @ w


def _forward_rows(rows_slice, feats, pair_rows, mask1, maskp, p, jnp, jax, axis_name):
    """Row-sharded forward. rows_slice: [ROWS] global row indices."""
    # atom conditioning, full 512 rows (cheap, replicated)
    atom_cond = feats @ p['w_atom_cond'] + p['b_atom_cond'] + p['pos_embed'][:N]

    ph = atom_cond @ p['pair_wh'] + p['pair_bh']          # [N,P]
    pw = atom_cond @ p['pair_ww'] + p['pair_bw']          # [N,P]

    # pair MLP on own rows: [ROWS, N, P]
    pf = pair_rows + ph[rows_slice][:, None, :] + pw[None, :, :]
    pf = jnp.maximum(pf @ p['pair_m1'] + p['pair_mb1'], 0.0) @ p['pair_m2'] + p['pair_mb2']

    bias_rows = (jnp.maximum(pf @ p['bias_w1'] + p['bias_b1'], 0.0)
                 @ p['bias_w2'] + p['bias_b2'])[..., 0]   # [ROWS, N]

    tp_rows = jnp.maximum(pf @ p['tp_w1'] + p['tp_b1'], 0.0) @ p['tp_w2'] + p['tp_b2']

    # mask: key mask AND query (block) mask for own rows
    attn_mask = mask1[None, None, :] & maskp[rows_slice][None, :, None]  # [1,ROWS,N]
    neg = np.float32(np.finfo(np.float32).min)
    scale = np.float32(HD ** -0.5)

    x = atom_cond  # full [N, D], replicated
    for l in range(NL):
        xr = x[rows_slice]                                 # [ROWS, D]
        m = jnp.mean(xr, axis=-1, keepdims=True)
        v = jnp.var(xr, axis=-1, keepdims=True)
        h = (xr - m) * jax.lax.rsqrt(v + 1e-5) * p['ln_g'][l] + p['ln_b'][l]
        h = _mm(h, p['t_w1'][l], jnp) + p['t_b1'][l]
        h = h * jax.nn.sigmoid(h)                          # silu
        h = _mm(h, p['t_w2'][l], jnp) + p['t_b2'][l]
        xr = xr + h

        # gather full post-transition x for K/V
        xg = jax.lax.all_gather(xr, axis_name, axis=0, tiled=True)  # [N, D]

        q = (_mm(xr, p['wq'][l], jnp) + p['bq'][l]).reshape(ROWS, H, HD).transpose(1, 0, 2)
        k = (_mm(xg, p['wk'][l], jnp) + p['bk'][l]).reshape(N, H, HD).transpose(1, 0, 2)
        vv = (_mm(xg, p['wv'][l], jnp) + p['bv'][l]).reshape(N, H, HD).transpose(1, 0, 2)

        aw = jnp.einsum('hqd,hkd->hqk', q, k) * scale + bias_rows[None, :, :]
        aw = jnp.where(attn_mask, aw, neg)
        probs = jax.nn.softmax(aw, axis=-1)
        o = jnp.einsum('hqk,hkd->hqd', probs, vv).transpose(1, 0, 2).reshape(ROWS, D)
        xr = _mm(o, p['wo'][l], jnp) + p['bo'][l]

        x = jax.lax.all_gather(xr, axis_name, axis=0, tiled=True)   # [N, D]

    ts_rows = x[rows_slice] @ p['w_ts'] + p['b_ts']        # [ROWS, TS]
    return x, ts_rows, tp_rows, bias_rows


_FN_CACHE = {}


def _get_fn():
    import jax
    import jax.numpy as jnp

    if 'fn' in _FN_CACHE:
        return _FN_CACHE['fn']

    devs = jax.devices()[:NCORES]
    assert len(devs) == NCORES

    def percore(rows_slice, pair_rows, feats_r, mask1_r, maskp_r, params_r):
        return _forward_rows(rows_slice, feats_r, pair_rows, mask1_r, maskp_r,
                             params_r, jnp, jax, 'c')

    fn = jax.pmap(percore, axis_name='c',
                  in_axes=(0, 0, None, None, None, None),
                  out_axes=(None, 0, 0, 0), devices=devs)
    _FN_CACHE['fn'] = fn
    return fn


def _kernel_device(atom_single_input_feats, atom_block_pair_input_feats,
                   atom_single_mask, atom_block_pair_mask,
                   atom_noised_coords, params):
    import jax
    import jax.numpy as jnp

    import ml_dtypes

    feats = np.asarray(atom_single_input_feats)[0]         # [N, TS]
    pair = np.asarray(atom_block_pair_input_feats)[0]      # [N, N, P]
    mask1 = np.asarray(atom_single_mask)[0]                # [N]
    maskp = np.asarray(atom_block_pair_mask)[0]            # [N]
    # big layer weights -> bf16: halves host->device transfer, native PE dtype
    BF16 = {'t_w1', 't_w2', 'wq', 'wk', 'wv', 'wo'}
    p = {k: (np.asarray(v).astype(ml_dtypes.bfloat16) if k in BF16
             else np.asarray(v))
         for k, v in params.items()}

    # per-core shards
    pair_sh = pair.reshape(NCORES, ROWS, N, P)
    rows_idx = np.arange(N, dtype=np.int32).reshape(NCORES, ROWS)

    fn = _get_fn()
    x, ts_sh, tp_sh, _ = fn(rows_idx, pair_sh, feats, mask1, maskp, p)

    x = np.asarray(x)                                      # [N, D]
    ts = np.asarray(ts_sh).reshape(N, TS)
    tp = np.asarray(tp_sh).reshape(N, N, P)
    return (x[None].astype(np.float32),
            ts[None].astype(np.float32),
            tp[None].astype(np.float32))


def _kernel_host(atom_single_input_feats, atom_block_pair_input_feats,
                 atom_single_mask, atom_block_pair_mask,
                 atom_noised_coords, params):
    """Pure numpy fallback — exact port of the reference."""
    feats = np.asarray(atom_single_input_feats, dtype=np.float32)
    pair_in = np.asarray(atom_block_pair_input_feats, dtype=np.float32)
    mask1 = np.asarray(atom_single_mask)
    maskp = np.asarray(atom_block_pair_mask)
    p = {k: np.asarray(v, dtype=np.float32) for k, v in params.items()}

    B, n, _ = feats.shape
    atom_cond = feats @ p['w_atom_cond'] + p['b_atom_cond'] + p['pos_embed'][:n]
    ph = atom_cond @ p['pair_wh'] + p['pair_bh']
    pw = atom_cond @ p['pair_ww'] + p['pair_bw']
    pf = pair_in + ph[:, :, None, :] + pw[:, None, :, :]
    pf = np.maximum(pf @ p['pair_m1'] + p['pair_mb1'], 0) @ p['pair_m2'] + p['pair_mb2']
    bias = (np.maximum(pf @ p['bias_w1'] + p['bias_b1'], 0) @ p['bias_w2'] + p['bias_b2'])[..., 0]
    attn_mask = (mask1[:, None, None, :] & maskp[:, None, :, None])
    neg = np.finfo(np.float32).min
    scale = HD ** -0.5
    x = atom_cond
    for l in range(NL):
        mu = x.mean(-1, keepdims=True)
        va = x.var(-1, keepdims=True)
        h = (x - mu) / np.sqrt(va + 1e-5) * p['ln_g'][l] + p['ln_b'][l]
        h1 = h @ p['t_w1'][l] + p['t_b1'][l]
        h1 = h1 / (1 + np.exp(-h1))
        x = x + h1 @ p['t_w2'][l] + p['t_b2'][l]
        q = (x @ p['wq'][l] + p['bq'][l]).reshape(B, n, H, HD).transpose(0, 2, 1, 3)
        k = (x @ p['wk'][l] + p['bk'][l]).reshape(B, n, H, HD).transpose(0, 2, 1, 3)
        v = (x @ p['wv'][l] + p['bv'][l]).reshape(B, n, H, HD).transpose(0, 2, 1, 3)
        aw = np.einsum('bhqd,bhkd->bhqk', q, k) * scale + bias[:, None, :, :]
        aw = np.where(attn_mask, aw, neg).astype(np.float32)
        aw = aw - aw.max(-1, keepdims=True)
        e = np.exp(aw)
        probs = e / e.sum(-1, keepdims=True)
        o = np.einsum('bhqk,bhkd->bhqd', probs, v).transpose(0, 2, 1, 3).reshape(B, n, D)
        x = o @ p['wo'][l] + p['bo'][l]
    ts_out = x @ p['w_ts'] + p['b_ts']
    tp_out = np.maximum(pf @ p['tp_w1'] + p['tp_b1'], 0) @ p['tp_w2'] + p['tp_b2']
    return (x.astype(np.float32), ts_out.astype(np.float32), tp_out.astype(np.float32))


def kernel(**inputs):
    try:
        return _kernel_device(**inputs)
    except Exception:
        import traceback
        traceback.print_exc()
        return _kernel_host(**inputs)


if __name__ == '__main__':
    pass
